# revision 1
# baseline (speedup 1.0000x reference)
"""Trainium2 Bass kernel for a dense transformer block (nn_Block_47888885351104).

Full inputs -> full outputs. Data-parallel: batch B=256 sharded across 8
NeuronCores (32 batches/core), one SPMD Bass/Tile program.

v3 design:
- fp8 e4m3 everywhere on the GEMM path: weights pre-scaled by WSCALE=64 into
  e4m3 normal range (descale folded into existing evac ops), activations
  (h, h2, att, a1) in e4m3. QKV/Wo/W1/W2 run entirely in DoubleRow perf mode
  (two 128-row K-tiles per instruction, 0.5 cyc/row): odd K chunks are padded
  with a zero slot, and biases ride along as an (e0-row ⊗ bias-row) DR pair,
  so there are no rank-1 bias matmuls and the W1 relu evac is bias-free and
  merged two chunks per instruction.
- One activation table (natural_log_exp_and_others, preloaded): LN rstd is
  Exp(-0.5*Ln(var*C/(C-1) + eps)); softmax Exp; Relu; Copy. Zero table swaps.
- LN apply fused into one dual-scalar DVE tensor_scalar (sub mean, mult rstd)
  per tile (all-SBUF => 2x DVE mode).
- Causal masking on gpsimd (Pool) affine_select per (batch, head-pair,
  region), emitted right after the producing Exp so AV never waits long.
- PSUM: one 2-bank "big" tag (W1 fc-pairs, score pairs, k/q) + two 1-bank
  tags, exactly 8 banks.
- Batch-pair iterations, software pipelined: MLP(i-1), attention(i) with A/B
  batches interleaved through the serial softmax/normalize chains, LN1/T1/
  QKV(i+1) all interleaved so no engine waits on a serial chain.
"""

import sys

sys.path.insert(0, "/opt/trn_rl_repo")

import os as _os

import numpy as np

import concourse.bass as bass
import concourse.mybir as mybir
import concourse.tile as tile
from concourse import bacc
from concourse.bass import ts
from concourse.bass_utils import run_bass_kernel_spmd
from concourse.masks import make_identity, make_upper_triangular

F32 = mybir.dt.float32
BF16 = mybir.dt.bfloat16
FP8A = mybir.dt.float8e4   # activations (e4m3)
FP8W = mybir.dt.float8e4   # weights (e4m3, pre-scaled by WSCALE)
WSCALE = 64.0
WINV = 1.0 / WSCALE

B, T, C, H, HS, FF = 256, 256, 384, 6, 64, 1536
NCORES = 8
NB = B // NCORES          # 32 batches per core
NPAIR = NB // 2           # 16 batch-pairs per core
NT = T // 128             # 2 token tiles per batch
NC = C // 128             # 3 channel chunks
NG = H // 2               # 3 head groups (2 heads of 64 share a 128 tile)
NF = FF // 128            # 12 hidden chunks
EPS = 1e-5
ISCALE = float(1.0 / np.sqrt(np.float32(C)))
VAR_SCALE = float(C / (C - 1))  # unbiased (ddof=1) variance correction

Act = mybir.ActivationFunctionType
Alu = mybir.AluOpType
DR = mybir.MatmulPerfMode.DoubleRow


def bcast(ap, n):
    """Broadcast the last (size-1) free dim of `ap` to n via a stride-0 AP."""
    new = [list(d) for d in ap.ap]
    assert new[-1][1] == 1
    new[-1] = [0, n]
    return bass.AP(ap.tensor, ap.offset, new)


def _emit(nc, tc, d):
    from contextlib import ExitStack

    ctx = ExitStack()
    g = lambda k, dflt: int(_os.environ.get(k, dflt))
    const = ctx.enter_context(tc.tile_pool(name="const", bufs=1))
    wpool = ctx.enter_context(tc.tile_pool(name="weights", bufs=1))
    stg = ctx.enter_context(tc.tile_pool(name="stg", bufs=1))
    xp = ctx.enter_context(tc.tile_pool(name="xp", bufs=g("XP_BUFS", 3)))
    sb = ctx.enter_context(tc.tile_pool(name="sb", bufs=g("SB_BUFS", 2)))
    sc = ctx.enter_context(tc.tile_pool(name="sc", bufs=g("SC_BUFS", 2)))
    # PSUM: "big" 2-bank slots x2 + "pb"/"small" 1-bank slots x2 = 8 banks.
    pBig = ctx.enter_context(tc.tile_pool(name="pBig", bufs=g("PBIG_BUFS", 2),
                                          space="PSUM"))
    pB = ctx.enter_context(tc.tile_pool(name="pB", bufs=g("PB_BUFS", 2),
                                        space="PSUM"))
    pS = ctx.enter_context(tc.tile_pool(name="pS", bufs=g("PS_BUFS", 2),
                                        space="PSUM"))

    # Preload the single activation table covering Exp/Ln/Relu/Copy/Identity
    # (natural_log_exp_and_others = id 6) so no further table loads appear.
    if g("K_TABLE", 1):
        nc.scalar.add_instruction(mybir.InstLoadActFuncSet(
            name=nc.get_next_instruction_name(), act_func_set_id=6,
            ins=[], outs=[]))

    ident = const.tile([128, 128], FP8A)
    make_identity(nc, ident[:])
    epst = const.tile([128, 1], F32, tag="epst")
    nc.vector.memset(epst[:], EPS)
    mask = const.tile([128, 128], BF16)  # mask[u, t] = 1 if t >= u else 0
    make_upper_triangular(nc, mask[:], val=1.0, diag=True)

    # ---- weights ---------------------------------------------------------
    # All GEMM weights are [128, 4(or 14), cols] fp8 with a zero/bias pad
    # slot so every contraction is a whole number of DoubleRow pairs.
    def stage(dram_ap, shape, tag):
        wf = stg.tile(shape, F32, tag="stage", name="stage_" + tag)
        nc.sync.dma_start(wf[:], dram_ap)
        return wf

    def load_qkv(dram_ap, tag, scale):
        wf = stg.tile([128, NC, H * HS], F32, tag="stage",
                      name="stage_" + tag)
        dram_v = dram_ap.rearrange("h (cc p) s -> p cc h s", p=128)
        for cc in range(NC):
            nc.sync.dma_start(
                wf[:, cc].rearrange("p (h s) -> p h s", h=H), dram_v[:, cc])
        wb = wpool.tile([128, NC + 1, H * HS], FP8W, tag=tag)
        nc.scalar.activation(wb[:, 0:NC], wf[:], Act.Copy, scale=scale)
        nc.vector.memset(wb[:, NC], 0.0)
        return wb

    wk = load_qkv(d["Wk"], "wk", WSCALE)
    wq = load_qkv(d["Wq"], "wq", ISCALE * WSCALE)
    wv = load_qkv(d["Wv"], "wv", WSCALE)

    def load_e0(dram_w, bias_ap, nchunk, cols, tag, bscale):
        """[128, nchunk+pad, cols] fp8: data chunks, then a bias slot whose
        row 0 holds bias*bscale (pairs with an e0-row on the rhs/lhs side),
        then a zero slot if nchunk is even (to keep slots a multiple of 2)."""
        nslot = nchunk + 1 + (1 - nchunk % 2)
        wf = stage(dram_w, [128, nchunk, cols], tag)
        wb = wpool.tile([128, nslot, cols], FP8W, tag=tag)
        nc.scalar.activation(wb[:, 0:nchunk], wf[:], Act.Copy, scale=WSCALE)
        nc.vector.memset(wb[:, nchunk:], 0.0)
        bf = stg.tile([1, cols], F32, tag="stage_b", name="stage_b_" + tag)
        nc.sync.dma_start(bf[:], bias_ap)
        nc.scalar.activation(wb[0:1, nchunk], bf[:], Act.Copy, scale=bscale)
        return wb

    wo = load_e0(d["Wo"].rearrange("(cc p) c2 -> p cc c2", p=128),
                 d["bo"][None, :], NC, C, "wo", WSCALE)
    w1 = load_e0(d["W1"].rearrange("(cc p) f -> p cc f", p=128),
                 d["b1"][None, :], NC, FF, "w1", WSCALE)
    w2 = load_e0(d["W2"].rearrange("(fc p) c2 -> p fc c2", p=128),
                 d["b2"][None, :], NF, C, "w2", WSCALE * WSCALE)
    x_d, out_d = d["x"], d["out"]

    # ---- helpers ---------------------------------------------------------
    pad_eng = nc.gpsimd if g("K_PADPOOL", 1) else nc.vector

    def pad_zero(dst_slot):
        pad_eng.memset(dst_slot, 0.0)

    def pad_e0(dst_slot):
        pad_eng.memset(dst_slot, 0.0)
        pad_eng.memset(dst_slot[0:1], 1.0)

    def ln_chain(xin, tag):
        """xin [128, 2, NT, C] f32 -> hc [128, 2, NT, C] FP8A."""
        st = sb.tile([128, 4, 6], F32, tag=tag + "st")
        mv = sb.tile([128, 4, 2], F32, tag=tag + "mv")
        for j in range(4):
            nc.vector.bn_stats(st[:, j], xin[:, j // 2, j % 2])
        for j in range(4):
            nc.vector.bn_aggr(mv[:, j], st[:, j])
        lnv = sb.tile([128, 4], F32, tag=tag + "lnv")
        nc.scalar.activation(lnv[:], mv[:, :, 1], Act.Ln,
                             bias=epst[:], scale=VAR_SCALE)
        r = sb.tile([128, 4], F32, tag=tag + "r")
        nc.scalar.activation(r[:], lnv[:], Act.Exp, scale=-0.5)
        hc = sb.tile([128, 2, NT, C], FP8A, tag=tag + "hc")
        for j in range(4):
            nc.vector.tensor_scalar(hc[:, j // 2, j % 2], xin[:, j // 2, j % 2],
                                    mv[:, j, 0:1], r[:, j:j + 1],
                                    Alu.subtract, Alu.mult)
        return hc

    def transpose_bt(hc, dst, b, tt, evac):
        """One (batch, token-tile) transpose: hc[:, b, tt] -> dst columns."""
        ps = pS.tile([128, NC, 128], F32, tag="ps")
        for cc in range(NC):
            nc.tensor.matmul(ps[:, cc], hc[:, b, tt, ts(cc, 128)],
                             ident[:], start=True, stop=True)
        dcol = b * T + tt * 128
        if evac == "act":
            nc.scalar.activation(dst[:, 0:NC, dcol:dcol + 128], ps[:],
                                 Act.Copy)
        else:
            nc.vector.tensor_copy(dst[:, 0:NC, dcol:dcol + 128], ps[:])

    def mm_dr(out_ps, lhs3, rhs3, nslot, stop=True):
        """out_ps (+)= sum over nslot 128-row K chunks, all DoubleRow."""
        npairs = nslot // 2
        assert nslot % 2 == 0
        for p in range(npairs):
            nc.tensor.matmul(out_ps, lhs3[:, 2 * p:2 * p + 2],
                             rhs3[:, 2 * p:2 * p + 2],
                             start=(p == 0), stop=(stop and p == npairs - 1),
                             perf_mode=DR)

    # ---- per-iteration stage emitters -----------------------------------
    state = {}

    def st_dma_in(i):
        xb = xp.tile([128, 2, NT, C], F32, tag="xb", bufs=g("XB_BUFS", 6))
        for b in range(2):
            nc.sync.dma_start(
                xb[:, b], x_d[2 * i + b].rearrange("(tt p) c -> p tt c", p=128))
        state["xb", i] = xb

    def st_ln1(i):
        state["hc", i] = ln_chain(state["xb", i], "ln1")

    def st_t1(i):
        hc = state["hc", i]
        hT = sb.tile([128, NC + 1, 2 * T], FP8A, tag="hT")
        pad_zero(hT[:, NC])
        for b in range(2):
            for tt in range(NT):
                transpose_bt(hc, hT, b, tt, _os.environ.get("E_T1", "act"))
        state["hT", i] = hT

    def st_qkv_kq(i):
        hT = state["hT", i]
        kT = sc.tile([128, NG, 2 * T], BF16, tag="kT")
        qT = sc.tile([128, NG, 2 * T], BF16, tag="qT")
        for gi in range(NG):
            k_ps = pBig.tile([128, 2 * T], F32, tag="big")
            mm_dr(k_ps[:], wk[:, :, ts(gi, 128)], hT, NC + 1)
            nc.scalar.activation(kT[:, gi], k_ps[:], Act.Copy, scale=WINV)
        for gi in range(NG):
            q_ps = pBig.tile([128, 2 * T], F32, tag="big")
            mm_dr(q_ps[:], wq[:, :, ts(gi, 128)], hT, NC + 1)
            nc.scalar.activation(qT[:, gi], q_ps[:], Act.Copy, scale=WINV)
        state["kT", i] = kT
        state["qT", i] = qT

    def st_qkv_v(i):
        hT = state.pop(("hT", i))
        vb = sc.tile([128, 2, NT, H, HS + 1], BF16, tag="vb", bufs=g("VB_BUFS", 2))
        nc.gpsimd.memset(vb[:, :, :, :, HS:HS + 1], 1.0)
        for b in range(2):
            for uc in range(NT):
                v_ps = pB.tile([128, C], F32, tag="pb")
                ucol = b * T + uc * 128
                mm_dr(v_ps[:], hT[:, :, ucol:ucol + 128], wv, NC + 1)
                if _os.environ.get("E_VB", "act") == "act":
                    nc.scalar.activation(
                        vb[:, b, uc, :, 0:HS],
                        v_ps[:].rearrange("p (h s) -> p h s", h=H),
                        Act.Copy, scale=WINV)
                else:
                    nc.vector.tensor_scalar_mul(
                        vb[:, b, uc, :, 0:HS],
                        v_ps[:].rearrange("p (h s) -> p h s", h=H), WINV)
        state["vb", i] = vb

    def st_scores(i, bs=(0, 1)):
        kT, qT = state["kT", i], state["qT", i]
        weis = state.get(("wei", i), [None, None])
        for b in bs:
            wei = sc.tile([128, H, 384], BF16, tag=f"wei{b}", name=f"wei{b}")
            tcol = b * T
            for gi in range(NG):
                # sub stride padded to 512 so each matmul's accumulation
                # group stays inside one 2KB PSUM bank.
                s_ps = pBig.tile([128, 2, 512], F32, tag="big")
                for sub in range(2):
                    nc.tensor.matmul(s_ps[:, sub, 0:256],
                                     qT[ts(sub, 64), gi, tcol:tcol + 128],
                                     kT[ts(sub, 64), gi, tcol:tcol + T],
                                     start=True, stop=True)
                    nc.tensor.matmul(s_ps[:, sub, 256:384],
                                     qT[ts(sub, 64), gi, tcol + 128:tcol + T],
                                     kT[ts(sub, 64), gi, tcol + 128:tcol + T],
                                     start=True, stop=True)
                nc.scalar.activation(wei[:, ts(gi, 2)], s_ps[:, :, 0:384],
                                     Act.Exp)
                # causal mask per region on the Pool engine, right behind
                # the producing Exp: region [0:128] is u0 x t0, region
                # [256:384] is u1 x t1.
                for lo in (0, 256):
                    nc.gpsimd.affine_select(
                        out=wei[:, ts(gi, 2), lo:lo + 128],
                        in_=wei[:, ts(gi, 2), lo:lo + 128],
                        compare_op=Alu.is_ge, fill=0.0, base=0,
                        pattern=[[0, 2], [1, 128]], channel_multiplier=-1)
            weis[b] = wei
        state["wei", i] = weis

    def st_av(i, b):
        wei, vb = state["wei", i][b], state["vb", i]
        o_pss = []
        for tt in range(NT):
            o_ps = pS.tile([128, H, HS + 1], F32, tag="ps")
            for h in range(H):
                if tt == 0:
                    nc.tensor.matmul(o_ps[:, h], wei[:, h, 0:128],
                                     vb[:, b, 0, h], start=True, stop=True)
                else:
                    nc.tensor.matmul(o_ps[:, h], wei[:, h, 128:256],
                                     vb[:, b, 0, h], start=True, stop=False)
                    nc.tensor.matmul(o_ps[:, h], wei[:, h, 256:384],
                                     vb[:, b, 1, h], start=False, stop=True)
            o_pss.append(o_ps)
        state["o_ps", i, b] = o_pss

    def st_nrm(i, b):
        o_pss = state.pop(("o_ps", i, b))
        att = sb.tile([128, NT, C], FP8A, tag=f"att{b}")
        for tt in range(NT):
            o_ps = o_pss[tt]
            rec = sb.tile([128, H, 1], F32, tag="rec")
            nc.vector.reciprocal(rec[:], o_ps[:, :, HS:HS + 1])
            nc.vector.tensor_tensor(
                out=att[:, tt].rearrange("p (h s) -> p h s", h=H),
                in0=o_ps[:, :, 0:HS], in1=bcast(rec[:], HS), op=Alu.mult)
        state["att", i, b] = att

    def st_t3(i, b, attT):
        att = state.pop(("att", i, b))
        for tt in range(NT):
            ps = pS.tile([128, NC, 128], F32, tag="ps")
            for cc in range(NC):
                nc.tensor.matmul(ps[:, cc], att[:, tt, ts(cc, 128)],
                                 ident[:], start=True, stop=True)
            dcol = b * T + tt * 128
            nc.vector.tensor_copy(attT[:, 0:NC, dcol:dcol + 128], ps[:])

    def st_wo(i, b, attT):
        xb = state["xb", i]
        y = state["y", i]
        for tt in range(NT):
            pr = pB.tile([128, C], F32, tag="pb")
            tcol = b * T + tt * 128
            mm_dr(pr[:], attT[:, :, tcol:tcol + 128], wo, NC + 1)
            nc.vector.scalar_tensor_tensor(
                out=y[:, b, tt], in0=pr[:], scalar=WINV,
                in1=xb[:, b, tt], op0=Alu.mult, op1=Alu.add)

    def st_ln2(i):
        state["hc2", i] = ln_chain(state["y", i], "ln2")

    def st_t2(i):
        hc2 = state["hc2", i]
        hT2 = sb.tile([128, NC + 1, 2 * T], FP8A, tag="hT2", bufs=4)
        pad_e0(hT2[:, NC])
        for b in range(2):
            for tt in range(NT):
                transpose_bt(hc2, hT2, b, tt, _os.environ.get("E_T2", "act"))
        state["hT2", i] = hT2

    def st_a1_alloc(i):
        a1 = sc.tile([128, NF + 2, 2 * T], FP8A, tag="a1", bufs=4)
        pad_e0(a1[:, NF])
        pad_zero(a1[:, NF + 1])
        state["a1", i] = a1

    def st_w1(i):
        hT2 = state.pop(("hT2", i))
        a1 = state["a1", i]
        for fq in range(NF // 2):
            a_ps = pBig.tile([128, 2, 2 * T], F32, tag="big")
            for j in range(2):
                mm_dr(a_ps[:, j], w1[:, :, ts(2 * fq + j, 128)], hT2, NC + 1)
            if fq % 2 == 0:
                nc.scalar.activation(a1[:, 2 * fq:2 * fq + 2], a_ps[:],
                                     Act.Relu)
            else:
                nc.vector.tensor_scalar_max(a1[:, 2 * fq:2 * fq + 2], a_ps[:],
                                            0.0)

    def st_w2_out(i, b):
        a1, y = state["a1", i], state["y", i]
        out_t = xp.tile([128, NT, C], F32, tag="outt")
        for tt in range(NT):
            ff_ps = pB.tile([128, C], F32, tag="pb")
            tcol = b * T + tt * 128
            mm_dr(ff_ps[:], a1[:, :, tcol:tcol + 128], w2, NF + 2)
            nc.vector.scalar_tensor_tensor(
                out=out_t[:, tt], in0=ff_ps[:], scalar=WINV * WINV,
                in1=y[:, b, tt], op0=Alu.mult, op1=Alu.add)
        nc.sync.dma_start(
            out_d[2 * i + b].rearrange("(tt p) c -> p tt c", p=128),
            out_t[:])

    # ---- software-pipelined main loop -----------------------------------
    PIPE = g("KERNEL_PIPE", 1)
    if not PIPE:
        for i in range(NPAIR):
            st_dma_in(i)
            st_ln1(i)
            st_t1(i)
            st_qkv_kq(i)
            st_qkv_v(i)
            st_scores(i)
            state["y", i] = xp.tile([128, 2, NT, C], F32, tag="y", name="y")
            attT = sb.tile([128, NC + 1, 2 * T], FP8A, tag="attT", name="attT")
            pad_e0(attT[:, NC])
            for b in range(2):
                st_av(i, b)
                st_nrm(i, b)
                st_t3(i, b, attT)
                st_wo(i, b, attT)
            st_ln2(i)
            st_a1_alloc(i)
            st_t2(i)
            st_w1(i)
            for b in range(2):
                st_w2_out(i, b)
    else:
        st_dma_in(0)
        st_dma_in(1)
        st_ln1(0)
        st_t1(0)
        st_qkv_kq(0)
        st_qkv_v(0)
        for i in range(NPAIR + 1):
            cur, prev, nxt = i, i - 1, i + 1
            if i + 2 <= NPAIR - 1:
                st_dma_in(i + 2)
            if nxt <= NPAIR - 1:
                st_ln1(nxt)
            if cur <= NPAIR - 1:
                st_scores(cur, bs=(0,))
            if prev >= 0:
                st_a1_alloc(prev)
                st_t2(prev)
            if cur <= NPAIR - 1:
                st_scores(cur, bs=(1,))
                state["y", cur] = xp.tile([128, 2, NT, C], F32, tag="y",
                                          name="y", bufs=g("Y_BUFS", 3))
                attT = sb.tile([128, NC + 1, 2 * T], FP8A, tag="attT",
                               name="attT")
                pad_e0(attT[:, NC])
                state["attT", cur] = attT
            if prev >= 0:
                st_w1(prev)
            if nxt <= NPAIR - 1:
                st_t1(nxt)
            if nxt <= NPAIR - 1:
                st_qkv_kq(nxt)
            if prev >= 0:
                st_w2_out(prev, 0)
            if cur <= NPAIR - 1:
                attT = state["attT", cur]
                st_av(cur, 0)
            if cur <= NPAIR - 1:
                st_nrm(cur, 0)
                st_t3(cur, 0, attT)
            if prev >= 0:
                st_w2_out(prev, 1)
            if cur <= NPAIR - 1:
                st_av(cur, 1)
            if nxt <= NPAIR - 1:
                st_qkv_v(nxt)
            if cur <= NPAIR - 1:
                st_nrm(cur, 1)
                st_t3(cur, 1, attT)
                st_wo(cur, 0, attT)
                st_wo(cur, 1, attT)
                st_ln2(cur)
            for key in list(state):
                if key[1] < i - 1:
                    del state[key]

    ctx.close()


def _build_program():
    nc = bacc.Bacc("TRN2", target_bir_lowering=False, debug=False)
    d = {}
    specs = {
        "x": [NB, T, C], "Wk": [H, C, HS], "Wq": [H, C, HS],
        "Wv": [H, C, HS], "Wo": [C, C], "bo": [C], "W1": [C, FF],
        "b1": [FF], "W2": [FF, C], "b2": [C],
    }
    for name, shape in specs.items():
        d[name] = nc.dram_tensor(name, shape, F32, kind="ExternalInput").ap()
    d["out"] = nc.dram_tensor("out", [NB, T, C], F32,
                              kind="ExternalOutput").ap()
    with tile.TileContext(nc) as tc:
        _emit(nc, tc, d)
    nc.compile()
    return nc


_CACHED_NC = None


def _get_program():
    global _CACHED_NC
    if _CACHED_NC is None:
        _CACHED_NC = _build_program()
    return _CACHED_NC


def kernel(**inputs):
    nc = _get_program()
    x = np.ascontiguousarray(np.asarray(inputs["x"], dtype=np.float32))
    shared = {k: np.ascontiguousarray(np.asarray(inputs[k], np.float32))
              for k in ("Wk", "Wq", "Wv", "Wo", "bo", "W1", "b1", "W2", "b2")}
    in_maps = [dict(shared, x=x[c * NB:(c + 1) * NB]) for c in range(NCORES)]
    res = run_bass_kernel_spmd(nc, in_maps, core_ids=list(range(NCORES)))
    out = np.concatenate([res.results[c]["out"] for c in range(NCORES)],
                         axis=0)
    return out.astype(np.float32)

